# revision 12
# baseline (speedup 1.0000x reference)
"""Trainium2 Bass kernel for nn_BoundaryPredictor2 (ragged_sequence).

Data-parallel over batch: each of the 8 NeuronCores processes one batch row
(B=8, L=1024, D=1024).

Per-core device pipeline (all fp32, matmuls streamed as float32r):
  1. Load hidden row as 8 tiles [128, 1024] (l on partitions, d on free),
     plus the same tiles shifted by one row (for adjacent-row dots).
  2. Row sum-of-squares via ACT Square+accum; adjacent-row dots via DVE
     scalar_tensor_tensor with accum_out.
  3. In the transposed [8, 128] domain (chunk on partitions, position on
     free axis, so +-1 position shifts are free-axis slices): cos_sim,
     p = clip((1-cos)/2, tiny, 1), and the boundary bit
     hard = (p + clip(u, tiny, 1) > 1) * mask.  This is exactly equivalent
     to the reference's sigmoid((logit(p)+logit(u))/T) > 0.5 route
     (verified bit-identical on the seeded inputs; margin ~2.4e-5).
  4. Segment ids hh1 = exclusive-cumsum(hard) via the native
     tensor_tensor_scan; cross-chunk offsets via a strict-lower-triangular
     [8,8] matmul; PE-transpose back to [128, 8] column layout.
  5. One-hot membership lel[l, s] = (hh1[l] == s) via iota + is_equal
     (only the lower-triangular windows s < 128*(chunk+1) can be nonzero).
  6. pooled_raw[s] = sum_l lel[l, s] * h[l] via PE matmuls (8 output windows
     of 128 segments; chunk lc only feeds windows m <= lc). Segment counts
     come from the same stationary with a ones moving vector.
  7. pooled = pooled_raw * 1/(n + 1e-9), applied during the PSUM->SBUF copy
     on the scalar engine (per-partition scale), then DMA out.

Host side: gather per-core boundary counts, compute the binomial loss with
float64 lgamma, and build short_mask = arange(L) < counts[:, None].
"""

import math
import sys

import numpy as np

try:
    import concourse.bass as bass  # noqa: F401
except ImportError:  # pragma: no cover
    sys.path.insert(0, "/opt/trn_rl_repo")

import concourse.bass as bass
import concourse.mybir as mybir
import concourse.tile as tile
from concourse.masks import make_identity

F32 = mybir.dt.float32
F32R = mybir.dt.float32r
Alu = mybir.AluOpType
Act = mybir.ActivationFunctionType

B, L, D = 8, 1024, 1024
P = 128
NCHUNK = L // P  # 8
PRIOR = 0.2
TINY = float(np.finfo(np.float32).tiny)

_NC_CACHE = {}


def _hoist_multiwaits(nc):
    """Walrus codegen for TRN2 only encodes a single embedded sem-wait per
    instruction ("Too many sync wait commands" otherwise). Split every
    multi-wait sync_info into standalone single-wait EventSemaphore
    instructions placed directly before the instruction on the same engine
    stream (the raw-bass wait_ge pattern)."""
    for f in nc.m.functions:
        for b in f.blocks:
            insts = list(b.instructions)
            if not any(
                i.sync_info is not None and len(i.sync_info.on_wait) > 1
                for i in insts
            ):
                continue
            new = []
            for i in insts:
                si = i.sync_info
                if si is not None and len(si.on_wait) > 1:
                    for k, w in enumerate(si.on_wait):
                        ev = mybir.InstEventSemaphore(
                            name=f"{i.name}-hw{k}",
                            ins=[],
                            outs=[],
                            sync_info=mybir.SyncInfo(on_wait=[w], on_update=[]),
                        )
                        ev.engine = i.engine
                        new.append(ev)
                    i.sync_info = mybir.SyncInfo(
                        on_wait=[], on_update=list(si.on_update)
                    )
                new.append(i)
            b.instructions = new


def build_nc(hoist=True):
    nc = bass.Bass("TRN2", target_bir_lowering=False)

    hidden_d = nc.dram_tensor("hidden", [L, D], F32, kind="ExternalInput")
    noise_d = nc.dram_tensor("noise", [L], F32, kind="ExternalInput")
    mask_d = nc.dram_tensor("mask", [L], F32, kind="ExternalInput")
    pooled_d = nc.dram_tensor("pooled", [L, D], F32, kind="ExternalOutput")
    stats_d = nc.dram_tensor("stats", [1], F32, kind="ExternalOutput")

    with tile.TileContext(nc) as tc:
        with (
            tc.tile_pool(name="hpool", bufs=1) as hpool,
            tc.tile_pool(name="shpool", bufs=2) as shpool,
            tc.tile_pool(name="lelpool", bufs=1) as lelpool,
            tc.tile_pool(name="spool", bufs=2) as spool,
            tc.tile_pool(name="small", bufs=1) as small,
            tc.tile_pool(name="outp", bufs=3) as outp,
            tc.tile_pool(name="pmain", bufs=2, space="PSUM") as pmain,
            tc.tile_pool(name="psmall", bufs=2, space="PSUM") as psmall,
            tc.tile_pool(name="ptr", bufs=2, space="PSUM") as ptr,
        ):
            # ---- constants / small buffers ----
            ident = small.tile([P, P], F32, tag="ident")
            make_identity(nc, ident[:])
            ones_col = small.tile([P, 1], F32, tag="ones_col")
            nc.vector.memset(ones_col[:], 1.0)
            # fp32r matmuls need even moving/dst element counts -> 2 cols
            ones2 = small.tile([P, 2], F32, tag="ones2")
            nc.vector.memset(ones2[:], 1.0)
            ones_col_r = small.tile([P, 2], F32R, tag="ones_col_r")
            nc.gpsimd.tensor_copy(ones_col_r[:], ones2[:])
            # strict-lower-triangular [8,8]: Lmat[k, m] = 1 if k < m else 0
            lmat = small.tile([NCHUNK, NCHUNK], F32, tag="lmat")
            nc.gpsimd.memset(lmat[:], 0.0)
            nc.gpsimd.affine_select(
                out=lmat[:],
                in_=lmat[:],
                compare_op=Alu.is_ge,
                fill=1.0,
                base=0,
                pattern=[[-1, NCHUNK]],
                channel_multiplier=1,
            )

            ss_cols = small.tile([P, NCHUNK], F32, tag="ss_cols")
            dot_cols = small.tile([P, NCHUNK], F32, tag="dot_cols")
            rt_cols = small.tile([P, NCHUNK], F32, tag="rt_cols")
            rn_cols = small.tile([P, NCHUNK], F32, tag="rn_cols")

            # ---- phase A: loads + row stats (position space, l on parts) ----
            h_tiles = []
            hr_tiles = []
            for lc in range(NCHUNK):
                h_t = hpool.tile([P, D], F32, tag=f"h{lc}")
                nc.sync.dma_start(h_t[:], hidden_d[P * lc : P * (lc + 1), :])
                h_tiles.append(h_t)
                # fp32r-rounded copy for the PE (fp32r streams at full rate)
                h_r = hpool.tile([P, D], F32R, tag=f"hr{lc}")
                nc.gpsimd.tensor_copy(h_r[:], h_t[:])
                hr_tiles.append(h_r)

            for lc in range(NCHUNK):
                h_t = h_tiles[lc]
                scr = spool.tile([P, D], F32, tag="scr")
                nc.vector.scalar_tensor_tensor(
                    out=scr[:],
                    in0=h_t[:],
                    scalar=1.0,
                    in1=h_t[:],
                    op0=Alu.mult,
                    op1=Alu.mult,
                    accum_out=ss_cols[:, lc : lc + 1],
                )
                # shifted-by-one-row copy of hidden for adjacent dots
                sh_t = shpool.tile([P, D], F32, tag="sh")
                if lc < NCHUNK - 1:
                    nc.sync.dma_start(
                        sh_t[:], hidden_d[P * lc + 1 : P * (lc + 1) + 1, :]
                    )
                else:
                    # last tile: only 127 shifted rows exist; zero row 127
                    nc.vector.memset(sh_t[96:P, :], 0.0)
                    nc.sync.dma_start(
                        sh_t[0 : P - 1, :], hidden_d[P * lc + 1 : L, :]
                    )
                scr2 = spool.tile([P, D], F32, tag="scr2")
                nc.vector.scalar_tensor_tensor(
                    out=scr2[:],
                    in0=h_t[:],
                    scalar=1.0,
                    in1=sh_t[:],
                    op0=Alu.mult,
                    op1=Alu.mult,
                    accum_out=dot_cols[:, lc : lc + 1],
                )

            # rn = 1 / max(sqrt(ss), 1e-12)   (position space [128, 8])
            nc.scalar.sqrt(rt_cols[:], ss_cols[:])
            nc.vector.tensor_scalar(
                out=rt_cols[:], in0=rt_cols[:], scalar1=1e-12, scalar2=None,
                op0=Alu.max,
            )
            nc.vector.reciprocal(rn_cols[:], rt_cols[:])

            # ---- phase B: transposed domain [8, 128] ----
            def transpose_to_T(src_cols, tag):
                ps = ptr.tile([NCHUNK, P], F32, tag="tps")
                nc.tensor.transpose(ps[:], src_cols[:], ident[:])
                dst = small.tile([NCHUNK, P], F32, tag=tag)
                nc.vector.tensor_copy(dst[:], ps[:])
                return dst

            rnT = transpose_to_T(rn_cols, "rnT")
            dotT = transpose_to_T(dot_cols, "dotT")

            # rn_next[i] = rn[i+1] (pair space): free shift + 7 cross elems
            rn_nextT = small.tile([NCHUNK, P], F32, tag="rn_nextT")
            nc.vector.tensor_copy(rn_nextT[:, 0 : P - 1], rnT[:, 1:P])
            nc.vector.memset(rn_nextT[:, P - 1 : P], 1.0)
            nc.sync.dma_start(
                rn_nextT[0 : NCHUNK - 1, P - 1 : P], rnT[1:NCHUNK, 0:1]
            )

            # cos = dot * rn * rn_next  (pair-index space)
            cosT = small.tile([NCHUNK, P], F32, tag="cosT")
            nc.vector.tensor_tensor(
                out=cosT[:], in0=dotT[:], in1=rnT[:], op=Alu.mult
            )
            nc.vector.tensor_tensor(
                out=cosT[:], in0=cosT[:], in1=rn_nextT[:], op=Alu.mult
            )
            # p = clip((1 - cos)/2, tiny, 1.0)
            pclT = small.tile([NCHUNK, P], F32, tag="pclT")
            nc.scalar.activation(
                pclT[:], cosT[:], Act.Copy, bias=0.5, scale=-0.5
            )
            nc.vector.tensor_scalar(
                out=pclT[:], in0=pclT[:], scalar1=TINY, scalar2=1.0,
                op0=Alu.max, op1=Alu.min,
            )

            # u, mask in transposed layout (natural contiguous loads)
            uT = small.tile([NCHUNK, P], F32, tag="uT")
            nc.sync.dma_start(uT[:], noise_d[:].rearrange("(c p) -> c p", p=P))
            maskT = small.tile([NCHUNK, P], F32, tag="maskT")
            nc.sync.dma_start(
                maskT[:], mask_d[:].rearrange("(c p) -> c p", p=P)
            )
            nc.vector.tensor_scalar(
                out=uT[:], in0=uT[:], scalar1=TINY, scalar2=1.0,
                op0=Alu.max, op1=Alu.min,
            )
            # u2[i] = u[i+1] (pair space)
            u2T = small.tile([NCHUNK, P], F32, tag="u2T")
            nc.vector.tensor_copy(u2T[:, 0 : P - 1], uT[:, 1:P])
            nc.vector.memset(u2T[:, P - 1 : P], 0.0)
            nc.sync.dma_start(
                u2T[0 : NCHUNK - 1, P - 1 : P], uT[1:NCHUNK, 0:1]
            )

            # h2[i] = (p[i] + u[i+1] > 1)  -> boundary bit at position i+1
            h2T = small.tile([NCHUNK, P], F32, tag="h2T")
            nc.vector.scalar_tensor_tensor(
                out=h2T[:], in0=pclT[:], scalar=-1.0, in1=u2T[:],
                op0=Alu.add, op1=Alu.add,
            )
            nc.vector.tensor_scalar(
                out=h2T[:], in0=h2T[:], scalar1=0.0, scalar2=None,
                op0=Alu.is_gt,
            )

            # hard[pos] = h2[pos-1] * mask[pos]; position 0 forced boundary
            hardT = small.tile([NCHUNK, P], F32, tag="hardT")
            nc.vector.tensor_copy(hardT[:, 1:P], h2T[:, 0 : P - 1])
            nc.vector.memset(hardT[:, 0:1], 0.0)
            nc.sync.dma_start(
                hardT[1:NCHUNK, 0:1], h2T[0 : NCHUNK - 1, P - 1 : P]
            )
            nc.vector.memset(hardT[0:1, 0:1], 1.0)
            nc.vector.tensor_tensor(
                out=hardT[:], in0=hardT[:], in1=maskT[:], op=Alu.mult
            )

            # cumsum within chunks; hh1 = inclusive - hard (exclusive)
            zerosT = small.tile([NCHUNK, P], F32, tag="zerosT")
            nc.vector.memset(zerosT[:], 0.0)
            csT = small.tile([NCHUNK, P], F32, tag="csT")
            nc.vector.tensor_tensor_scan(
                csT[:], hardT[:], zerosT[:], 0.0, op0=Alu.add, op1=Alu.add
            )
            hh1T = small.tile([NCHUNK, P], F32, tag="hh1T")
            nc.vector.tensor_tensor(
                out=hh1T[:], in0=csT[:], in1=hardT[:], op=Alu.subtract
            )

            # cross-chunk offsets: offs[c] = sum_{k<c} totals[k]
            offs_ps = ptr.tile([NCHUNK, 1], F32, tag="tps")
            nc.tensor.matmul(
                offs_ps[:], lmat[:], csT[:, P - 1 : P], start=True, stop=True
            )
            offs_col = small.tile([NCHUNK, 1], F32, tag="offs_col")
            nc.vector.tensor_copy(offs_col[:], offs_ps[:])
            hh1gT = small.tile([NCHUNK, P], F32, tag="hh1gT")
            nc.vector.tensor_scalar(
                out=hh1gT[:], in0=hh1T[:], scalar1=offs_col[:], scalar2=None,
                op0=Alu.add,
            )

            # total boundary count -> stats
            nb_ps = ptr.tile([1, 1], F32, tag="tps")
            nc.tensor.matmul(
                nb_ps[:], ones_col[0:NCHUNK, 0:1], csT[:, P - 1 : P],
                start=True, stop=True,
            )
            nbt = small.tile([1, 1], F32, tag="nbt")
            nc.vector.tensor_copy(nbt[:], nb_ps[:])
            nc.sync.dma_start(stats_d[:], nbt[:])

            # back to column layout [128, 8]
            hh1_ps = ptr.tile([P, NCHUNK], F32, tag="tps")
            nc.tensor.transpose(
                hh1_ps[:], hh1gT[:], ident[0:NCHUNK, 0:NCHUNK]
            )
            hh1g = small.tile([P, NCHUNK], F32, tag="hh1g")
            nc.vector.tensor_copy(hh1g[:], hh1_ps[:])

            # ---- phase C: one-hot membership + pooling matmuls ----
            iota_b = small.tile([P, D], F32, tag="iota_b")
            nc.gpsimd.iota(
                iota_b[:],
                pattern=[[1, D]],
                base=0,
                channel_multiplier=0,
                allow_small_or_imprecise_dtypes=True,
            )
            lel_tiles = []
            for lc in range(NCHUNK):
                W = P * (lc + 1)
                lel_t = lelpool.tile([P, L], F32R, tag=f"lel{lc}")
                nc.vector.tensor_scalar(
                    out=lel_t[:, 0:W], in0=iota_b[:, 0:W],
                    scalar1=hh1g[:, lc : lc + 1], scalar2=None,
                    op0=Alu.is_equal,
                )
                lel_tiles.append(lel_t)

            for m in range(NCHUNK):
                psum = pmain.tile([P, D], F32, tag="psum")
                npsum = psmall.tile([P, 2], F32, tag="npsum")
                for lc in range(m, NCHUNK):
                    st = lc == m
                    sp = lc == NCHUNK - 1
                    lelT = lel_tiles[lc][:, P * m : P * (m + 1)]
                    h_r = hr_tiles[lc][:]
                    nc.tensor.matmul(
                        psum[:, 0:512], lelT, h_r[:, 0:512], start=st, stop=sp
                    )
                    nc.tensor.matmul(
                        psum[:, 512:1024], lelT, h_r[:, 512:1024],
                        start=st, stop=sp,
                    )
                    nc.tensor.matmul(
                        npsum[:], lelT, ones_col_r[:], start=st, stop=sp,
                    )
                ncol = small.tile([P, 1], F32, tag="ncol")
                nc.vector.tensor_scalar(
                    out=ncol[:], in0=npsum[:, 0:1], scalar1=1e-9, scalar2=None,
                    op0=Alu.add,
                )
                rcol = small.tile([P, 1], F32, tag="rcol")
                nc.vector.reciprocal(rcol[:], ncol[:])
                outt = outp.tile([P, D], F32, tag="outt")
                nc.scalar.activation(
                    outt[:], psum[:], Act.Copy, bias=0.0, scale=rcol[:]
                )
                nc.sync.dma_start(pooled_d[P * m : P * (m + 1), :], outt[:])

    if hoist:
        _hoist_multiwaits(nc)
    return nc


def get_nc(hoist=True):
    key = ("nc", hoist)
    if key not in _NC_CACHE:
        _NC_CACHE[key] = build_nc(hoist=hoist)
    return _NC_CACHE[key]


def _binomial_loss_host(n, total, prior=PRIOR):
    n = float(n)
    total = float(total)
    logp = (
        math.lgamma(total + 1.0)
        - math.lgamma(n + 1.0)
        - math.lgamma(total - n + 1.0)
        + n * math.log(prior)
        + (total - n) * math.log1p(-prior)
    )
    return np.float32(-logp / total)


def _reference_numpy(hidden, attention_mask, noise_u, Wq, Wk):
    """Pure-numpy replica of the oracle; only used if Wq/Wk are not identity
    (the shipped problem always uses identity projections)."""
    h = hidden.astype(np.float64)
    n = np.maximum(np.linalg.norm(h, axis=-1, keepdims=True), 1e-12)
    hn = h / n
    q = hn[:, :-1] @ Wq.T.astype(np.float64)
    k = hn[:, 1:] @ Wk.T.astype(np.float64)
    cos = np.einsum("bld,bld->bl", q, k)
    probs = np.clip((1.0 - cos) / 2.0, 0.0, 1.0)
    probs = np.concatenate([np.ones((B, 1)), probs], axis=1)
    tiny = float(np.finfo(np.float32).tiny)
    p = np.clip(probs, tiny, 1.0 - 1e-30)
    u = np.clip(noise_u.astype(np.float64), tiny, 1.0 - 1e-12)
    logits = np.log(p) - np.log1p(-p) + np.log(u) - np.log1p(-u)
    hard = (logits > 0.0).astype(np.float32) * attention_mask
    hh1 = np.cumsum(hard, axis=1) - hard
    s_range = np.arange(L, dtype=np.float32)
    pooled = np.zeros((B, L, D), np.float32)
    for b in range(B):
        lel = (s_range[None, :] == hh1[b][:, None]).astype(np.float32)
        bar = lel / (lel.sum(axis=0, keepdims=True) + 1e-9)
        pooled[b] = (hidden[b].T @ bar).T
    nb = hard.sum()
    tp = attention_mask.sum()
    loss = _binomial_loss_host(nb, tp)
    counts = (hard == 1.0).sum(axis=1)
    short_mask = (np.arange(L)[None, :] < counts[:, None]).astype(np.float32)
    return (pooled, loss, np.float32(nb), np.float32(tp), short_mask)


def kernel(hidden, attention_mask, noise_u, Wq=None, Wk=None, **_unused):
    hidden = np.ascontiguousarray(np.asarray(hidden, dtype=np.float32))
    attention_mask = np.ascontiguousarray(
        np.asarray(attention_mask, dtype=np.float32)
    )
    noise_u = np.ascontiguousarray(np.asarray(noise_u, dtype=np.float32))

    eye = np.eye(D, dtype=np.float32)
    for W in (Wq, Wk):
        if W is not None and not np.array_equal(np.asarray(W), eye):
            return _reference_numpy(
                hidden, attention_mask, noise_u, np.asarray(Wq), np.asarray(Wk)
            )

    from concourse.bass_utils import run_bass_kernel_spmd

    nc = get_nc()
    in_maps = [
        {
            "hidden": hidden[b],
            "noise": noise_u[b],
            "mask": attention_mask[b],
        }
        for b in range(B)
    ]
    res = run_bass_kernel_spmd(nc, in_maps, core_ids=list(range(B)))

    pooled = np.stack([res.results[b]["pooled"] for b in range(B)], axis=0)
    counts = np.array(
        [float(res.results[b]["stats"][0]) for b in range(B)], dtype=np.float64
    )
    nb = counts.sum()
    tp = float(attention_mask.sum(dtype=np.float64))
    loss = _binomial_loss_host(nb, tp)
    short_mask = (
        np.arange(L, dtype=np.float32)[None, :] < counts[:, None]
    ).astype(np.float32)
    return (pooled, loss, np.float32(nb), np.float32(tp), short_mask)


if __name__ == "__main__":
    rng = np.random.default_rng(0)
    h = rng.standard_normal((B, L, D), dtype=np.float32)
    m = np.ones((B, L), np.float32)
    u = rng.random((B, L), dtype=np.float32)
    out = kernel(h, m, u, np.eye(D, np.float32), np.eye(D, np.float32))
    print([np.asarray(o).shape for o in out])


# revision 13
# speedup vs baseline: 1.3055x; 1.3055x over previous
"""Trainium2 Bass kernel for nn_BoundaryPredictor2 (ragged_sequence).

Data-parallel over batch: each of the 8 NeuronCores processes one batch row
(B=8, L=1024, D=1024).

Per-core device pipeline (all fp32, matmuls streamed as float32r):
  1. Load hidden row as 8 tiles [128, 1024] (l on partitions, d on free),
     plus the same tiles shifted by one row (for adjacent-row dots).
  2. Row sum-of-squares via ACT Square+accum; adjacent-row dots via DVE
     scalar_tensor_tensor with accum_out.
  3. In the transposed [8, 128] domain (chunk on partitions, position on
     free axis, so +-1 position shifts are free-axis slices): cos_sim,
     p = clip((1-cos)/2, tiny, 1), and the boundary bit
     hard = (p + clip(u, tiny, 1) > 1) * mask.  This is exactly equivalent
     to the reference's sigmoid((logit(p)+logit(u))/T) > 0.5 route
     (verified bit-identical on the seeded inputs; margin ~2.4e-5).
  4. Segment ids hh1 = exclusive-cumsum(hard) via the native
     tensor_tensor_scan; cross-chunk offsets via a strict-lower-triangular
     [8,8] matmul; PE-transpose back to [128, 8] column layout.
  5. One-hot membership lel[l, s] = (hh1[l] == s) via iota + is_equal
     (only the lower-triangular windows s < 128*(chunk+1) can be nonzero).
  6. pooled_raw[s] = sum_l lel[l, s] * h[l] via PE matmuls (8 output windows
     of 128 segments; chunk lc only feeds windows m <= lc). Segment counts
     come from the same stationary with a ones moving vector.
  7. pooled = pooled_raw * 1/(n + 1e-9), applied during the PSUM->SBUF copy
     on the scalar engine (per-partition scale), then DMA out.

Host side: gather per-core boundary counts, compute the binomial loss with
float64 lgamma, and build short_mask = arange(L) < counts[:, None].
"""

import math
import sys

import numpy as np

try:
    import concourse.bass as bass  # noqa: F401
except ImportError:  # pragma: no cover
    sys.path.insert(0, "/opt/trn_rl_repo")

import concourse.bass as bass
import concourse.mybir as mybir
import concourse.tile as tile
from concourse.masks import make_identity

F32 = mybir.dt.float32
F32R = mybir.dt.float32r
Alu = mybir.AluOpType
Act = mybir.ActivationFunctionType

B, L, D = 8, 1024, 1024
P = 128
NCHUNK = L // P  # 8
PRIOR = 0.2
TINY = float(np.finfo(np.float32).tiny)

_NC_CACHE = {}


def _hoist_multiwaits(nc):
    """Walrus codegen for TRN2 only encodes a single embedded sem-wait per
    instruction ("Too many sync wait commands" otherwise). Split every
    multi-wait sync_info into standalone single-wait EventSemaphore
    instructions placed directly before the instruction on the same engine
    stream (the raw-bass wait_ge pattern)."""
    for f in nc.m.functions:
        for b in f.blocks:
            insts = list(b.instructions)
            if not any(
                i.sync_info is not None and len(i.sync_info.on_wait) > 1
                for i in insts
            ):
                continue
            new = []
            for i in insts:
                si = i.sync_info
                if si is not None and len(si.on_wait) > 1:
                    for k, w in enumerate(si.on_wait):
                        ev = mybir.InstEventSemaphore(
                            name=f"{i.name}-hw{k}",
                            ins=[],
                            outs=[],
                            sync_info=mybir.SyncInfo(on_wait=[w], on_update=[]),
                        )
                        ev.engine = i.engine
                        new.append(ev)
                    i.sync_info = mybir.SyncInfo(
                        on_wait=[], on_update=list(si.on_update)
                    )
                new.append(i)
            b.instructions = new


def build_nc(hoist=True):
    nc = bass.Bass("TRN2", target_bir_lowering=False)

    hidden_d = nc.dram_tensor("hidden", [L, D], F32, kind="ExternalInput")
    noise_d = nc.dram_tensor("noise", [L], F32, kind="ExternalInput")
    mask_d = nc.dram_tensor("mask", [L], F32, kind="ExternalInput")
    pooled_d = nc.dram_tensor("pooled", [L, D], F32, kind="ExternalOutput")
    stats_d = nc.dram_tensor("stats", [1], F32, kind="ExternalOutput")

    with tile.TileContext(nc) as tc:
        with (
            tc.tile_pool(name="hpool", bufs=1) as hpool,
            tc.tile_pool(name="shpool", bufs=2) as shpool,
            tc.tile_pool(name="lelpool", bufs=1) as lelpool,
            tc.tile_pool(name="spool", bufs=2) as spool,
            tc.tile_pool(name="small", bufs=1) as small,
            tc.tile_pool(name="outp", bufs=3) as outp,
            tc.tile_pool(name="pmain", bufs=2, space="PSUM") as pmain,
            tc.tile_pool(name="psmall", bufs=2, space="PSUM") as psmall,
            tc.tile_pool(name="ptr", bufs=2, space="PSUM") as ptr,
        ):
            # ---- constants / small buffers ----
            ident = small.tile([P, P], F32, tag="ident")
            make_identity(nc, ident[:])
            ones_col = small.tile([P, 1], F32, tag="ones_col")
            nc.vector.memset(ones_col[:], 1.0)
            # fp32r matmuls need even moving/dst element counts -> 2 cols
            ones2 = small.tile([P, 2], F32, tag="ones2")
            nc.vector.memset(ones2[:], 1.0)
            ones_col_r = small.tile([P, 2], F32R, tag="ones_col_r")
            nc.scalar.activation(ones_col_r[:], ones2[:], Act.Copy, bias=0.0)
            # strict-lower-triangular [8,8]: Lmat[k, m] = 1 if k < m else 0
            lmat = small.tile([NCHUNK, NCHUNK], F32, tag="lmat")
            nc.gpsimd.memset(lmat[:], 0.0)
            nc.gpsimd.affine_select(
                out=lmat[:],
                in_=lmat[:],
                compare_op=Alu.is_ge,
                fill=1.0,
                base=0,
                pattern=[[-1, NCHUNK]],
                channel_multiplier=1,
            )

            ss_cols = small.tile([P, NCHUNK], F32, tag="ss_cols")
            dot_cols = small.tile([P, NCHUNK], F32, tag="dot_cols")
            rt_cols = small.tile([P, NCHUNK], F32, tag="rt_cols")
            rn_cols = small.tile([P, NCHUNK], F32, tag="rn_cols")

            # ---- phase A: loads + row stats (position space, l on parts) ----
            h_tiles = []
            hr_tiles = []
            for lc in range(NCHUNK):
                h_t = hpool.tile([P, D], F32, tag=f"h{lc}")
                nc.sync.dma_start(h_t[:], hidden_d[P * lc : P * (lc + 1), :])
                h_tiles.append(h_t)
                # fp32r-rounded copy for the PE (fp32r streams at full rate)
                h_r = hpool.tile([P, D], F32R, tag=f"hr{lc}")
                nc.scalar.activation(h_r[:], h_t[:], Act.Copy, bias=0.0)
                hr_tiles.append(h_r)

            for lc in range(NCHUNK):
                h_t = h_tiles[lc]
                scr = spool.tile([P, D], F32, tag="scr")
                nc.scalar.activation(
                    scr[:],
                    h_t[:],
                    Act.Square,
                    accum_out=ss_cols[:, lc : lc + 1],
                )
                # shifted-by-one-row copy of hidden for adjacent dots
                sh_t = shpool.tile([P, D], F32, tag="sh")
                if lc < NCHUNK - 1:
                    nc.sync.dma_start(
                        sh_t[:], hidden_d[P * lc + 1 : P * (lc + 1) + 1, :]
                    )
                else:
                    # last tile: only 127 shifted rows exist; zero row 127
                    nc.vector.memset(sh_t[96:P, :], 0.0)
                    nc.sync.dma_start(
                        sh_t[0 : P - 1, :], hidden_d[P * lc + 1 : L, :]
                    )
                scr2 = spool.tile([P, D], F32, tag="scr2")
                nc.vector.tensor_tensor(
                    out=scr2[:], in0=h_t[:], in1=sh_t[:], op=Alu.mult
                )
                nc.vector.tensor_reduce(
                    out=dot_cols[:, lc : lc + 1], in_=scr2[:],
                    axis=mybir.AxisListType.X, op=Alu.add,
                )

            # rn = 1 / max(sqrt(ss), 1e-12)   (position space [128, 8])
            nc.scalar.sqrt(rt_cols[:], ss_cols[:])
            nc.vector.tensor_scalar(
                out=rt_cols[:], in0=rt_cols[:], scalar1=1e-12, scalar2=None,
                op0=Alu.max,
            )
            nc.vector.reciprocal(rn_cols[:], rt_cols[:])

            # ---- phase B: transposed domain [8, 128] ----
            def transpose_to_T(src_cols, tag):
                ps = ptr.tile([NCHUNK, P], F32, tag="tps")
                nc.tensor.transpose(ps[:], src_cols[:], ident[:])
                dst = small.tile([NCHUNK, P], F32, tag=tag)
                nc.vector.tensor_copy(dst[:], ps[:])
                return dst

            rnT = transpose_to_T(rn_cols, "rnT")
            dotT = transpose_to_T(dot_cols, "dotT")

            # rn_next[i] = rn[i+1] (pair space): free shift + 7 cross elems
            rn_nextT = small.tile([NCHUNK, P], F32, tag="rn_nextT")
            nc.vector.tensor_copy(rn_nextT[:, 0 : P - 1], rnT[:, 1:P])
            nc.vector.memset(rn_nextT[:, P - 1 : P], 1.0)
            nc.sync.dma_start(
                rn_nextT[0 : NCHUNK - 1, P - 1 : P], rnT[1:NCHUNK, 0:1]
            )

            # cos = dot * rn * rn_next  (pair-index space)
            cosT = small.tile([NCHUNK, P], F32, tag="cosT")
            nc.vector.tensor_tensor(
                out=cosT[:], in0=dotT[:], in1=rnT[:], op=Alu.mult
            )
            nc.vector.tensor_tensor(
                out=cosT[:], in0=cosT[:], in1=rn_nextT[:], op=Alu.mult
            )
            # p = clip((1 - cos)/2, tiny, 1.0)
            pclT = small.tile([NCHUNK, P], F32, tag="pclT")
            nc.scalar.activation(
                pclT[:], cosT[:], Act.Copy, bias=0.5, scale=-0.5
            )
            nc.vector.tensor_scalar(
                out=pclT[:], in0=pclT[:], scalar1=TINY, scalar2=1.0,
                op0=Alu.max, op1=Alu.min,
            )

            # u, mask in transposed layout (natural contiguous loads)
            uT = small.tile([NCHUNK, P], F32, tag="uT")
            nc.sync.dma_start(uT[:], noise_d[:].rearrange("(c p) -> c p", p=P))
            maskT = small.tile([NCHUNK, P], F32, tag="maskT")
            nc.sync.dma_start(
                maskT[:], mask_d[:].rearrange("(c p) -> c p", p=P)
            )
            nc.vector.tensor_scalar(
                out=uT[:], in0=uT[:], scalar1=TINY, scalar2=1.0,
                op0=Alu.max, op1=Alu.min,
            )
            # u2[i] = u[i+1] (pair space)
            u2T = small.tile([NCHUNK, P], F32, tag="u2T")
            nc.vector.tensor_copy(u2T[:, 0 : P - 1], uT[:, 1:P])
            nc.vector.memset(u2T[:, P - 1 : P], 0.0)
            nc.sync.dma_start(
                u2T[0 : NCHUNK - 1, P - 1 : P], uT[1:NCHUNK, 0:1]
            )

            # h2[i] = (p[i] + u[i+1] > 1)  -> boundary bit at position i+1
            h2T = small.tile([NCHUNK, P], F32, tag="h2T")
            nc.vector.scalar_tensor_tensor(
                out=h2T[:], in0=pclT[:], scalar=-1.0, in1=u2T[:],
                op0=Alu.add, op1=Alu.add,
            )
            nc.vector.tensor_scalar(
                out=h2T[:], in0=h2T[:], scalar1=0.0, scalar2=None,
                op0=Alu.is_gt,
            )

            # hard[pos] = h2[pos-1] * mask[pos]; position 0 forced boundary
            hardT = small.tile([NCHUNK, P], F32, tag="hardT")
            nc.vector.tensor_copy(hardT[:, 1:P], h2T[:, 0 : P - 1])
            nc.vector.memset(hardT[:, 0:1], 0.0)
            nc.sync.dma_start(
                hardT[1:NCHUNK, 0:1], h2T[0 : NCHUNK - 1, P - 1 : P]
            )
            nc.vector.memset(hardT[0:1, 0:1], 1.0)
            nc.vector.tensor_tensor(
                out=hardT[:], in0=hardT[:], in1=maskT[:], op=Alu.mult
            )

            # cumsum within chunks; hh1 = inclusive - hard (exclusive)
            zerosT = small.tile([NCHUNK, P], F32, tag="zerosT")
            nc.vector.memset(zerosT[:], 0.0)
            csT = small.tile([NCHUNK, P], F32, tag="csT")
            nc.vector.tensor_tensor_scan(
                csT[:], hardT[:], zerosT[:], 0.0, op0=Alu.add, op1=Alu.add
            )
            hh1T = small.tile([NCHUNK, P], F32, tag="hh1T")
            nc.vector.tensor_tensor(
                out=hh1T[:], in0=csT[:], in1=hardT[:], op=Alu.subtract
            )

            # cross-chunk offsets: offs[c] = sum_{k<c} totals[k]
            offs_ps = ptr.tile([NCHUNK, 1], F32, tag="tps")
            nc.tensor.matmul(
                offs_ps[:], lmat[:], csT[:, P - 1 : P], start=True, stop=True
            )
            offs_col = small.tile([NCHUNK, 1], F32, tag="offs_col")
            nc.vector.tensor_copy(offs_col[:], offs_ps[:])
            hh1gT = small.tile([NCHUNK, P], F32, tag="hh1gT")
            nc.vector.tensor_scalar(
                out=hh1gT[:], in0=hh1T[:], scalar1=offs_col[:], scalar2=None,
                op0=Alu.add,
            )

            # total boundary count -> stats
            nb_ps = ptr.tile([1, 1], F32, tag="tps")
            nc.tensor.matmul(
                nb_ps[:], ones_col[0:NCHUNK, 0:1], csT[:, P - 1 : P],
                start=True, stop=True,
            )
            nbt = small.tile([1, 1], F32, tag="nbt")
            nc.vector.tensor_copy(nbt[:], nb_ps[:])
            nc.sync.dma_start(stats_d[:], nbt[:])

            # back to column layout [128, 8]
            hh1_ps = ptr.tile([P, NCHUNK], F32, tag="tps")
            nc.tensor.transpose(
                hh1_ps[:], hh1gT[:], ident[0:NCHUNK, 0:NCHUNK]
            )
            hh1g = small.tile([P, NCHUNK], F32, tag="hh1g")
            nc.vector.tensor_copy(hh1g[:], hh1_ps[:])

            # ---- phase C: one-hot membership + pooling matmuls ----
            iota_b = small.tile([P, D], F32, tag="iota_b")
            nc.gpsimd.iota(
                iota_b[:],
                pattern=[[1, D]],
                base=0,
                channel_multiplier=0,
                allow_small_or_imprecise_dtypes=True,
            )
            lel_tiles = []
            for lc in range(NCHUNK):
                W = P * (lc + 1)
                lel_t = lelpool.tile([P, L], F32R, tag=f"lel{lc}")
                nc.vector.tensor_scalar(
                    out=lel_t[:, 0:W], in0=iota_b[:, 0:W],
                    scalar1=hh1g[:, lc : lc + 1], scalar2=None,
                    op0=Alu.is_equal,
                )
                lel_tiles.append(lel_t)

            for m in range(NCHUNK):
                psum = pmain.tile([P, D], F32, tag="psum")
                npsum = psmall.tile([P, 2], F32, tag="npsum")
                for lc in range(m, NCHUNK):
                    st = lc == m
                    sp = lc == NCHUNK - 1
                    lelT = lel_tiles[lc][:, P * m : P * (m + 1)]
                    h_r = hr_tiles[lc][:]
                    nc.tensor.matmul(
                        psum[:, 0:512], lelT, h_r[:, 0:512], start=st, stop=sp
                    )
                    nc.tensor.matmul(
                        psum[:, 512:1024], lelT, h_r[:, 512:1024],
                        start=st, stop=sp,
                    )
                    nc.tensor.matmul(
                        npsum[:], lelT, ones_col_r[:], start=st, stop=sp,
                    )
                ncol = small.tile([P, 1], F32, tag="ncol")
                nc.vector.tensor_scalar(
                    out=ncol[:], in0=npsum[:, 0:1], scalar1=1e-9, scalar2=None,
                    op0=Alu.add,
                )
                rcol = small.tile([P, 1], F32, tag="rcol")
                nc.vector.reciprocal(rcol[:], ncol[:])
                outt = outp.tile([P, D], F32, tag="outt")
                nc.scalar.activation(
                    outt[:], psum[:], Act.Copy, bias=0.0, scale=rcol[:]
                )
                nc.sync.dma_start(pooled_d[P * m : P * (m + 1), :], outt[:])

    if hoist:
        _hoist_multiwaits(nc)
    return nc


def get_nc(hoist=True):
    key = ("nc", hoist)
    if key not in _NC_CACHE:
        _NC_CACHE[key] = build_nc(hoist=hoist)
    return _NC_CACHE[key]


def _binomial_loss_host(n, total, prior=PRIOR):
    n = float(n)
    total = float(total)
    logp = (
        math.lgamma(total + 1.0)
        - math.lgamma(n + 1.0)
        - math.lgamma(total - n + 1.0)
        + n * math.log(prior)
        + (total - n) * math.log1p(-prior)
    )
    return np.float32(-logp / total)


def _reference_numpy(hidden, attention_mask, noise_u, Wq, Wk):
    """Pure-numpy replica of the oracle; only used if Wq/Wk are not identity
    (the shipped problem always uses identity projections)."""
    h = hidden.astype(np.float64)
    n = np.maximum(np.linalg.norm(h, axis=-1, keepdims=True), 1e-12)
    hn = h / n
    q = hn[:, :-1] @ Wq.T.astype(np.float64)
    k = hn[:, 1:] @ Wk.T.astype(np.float64)
    cos = np.einsum("bld,bld->bl", q, k)
    probs = np.clip((1.0 - cos) / 2.0, 0.0, 1.0)
    probs = np.concatenate([np.ones((B, 1)), probs], axis=1)
    tiny = float(np.finfo(np.float32).tiny)
    p = np.clip(probs, tiny, 1.0 - 1e-30)
    u = np.clip(noise_u.astype(np.float64), tiny, 1.0 - 1e-12)
    logits = np.log(p) - np.log1p(-p) + np.log(u) - np.log1p(-u)
    hard = (logits > 0.0).astype(np.float32) * attention_mask
    hh1 = np.cumsum(hard, axis=1) - hard
    s_range = np.arange(L, dtype=np.float32)
    pooled = np.zeros((B, L, D), np.float32)
    for b in range(B):
        lel = (s_range[None, :] == hh1[b][:, None]).astype(np.float32)
        bar = lel / (lel.sum(axis=0, keepdims=True) + 1e-9)
        pooled[b] = (hidden[b].T @ bar).T
    nb = hard.sum()
    tp = attention_mask.sum()
    loss = _binomial_loss_host(nb, tp)
    counts = (hard == 1.0).sum(axis=1)
    short_mask = (np.arange(L)[None, :] < counts[:, None]).astype(np.float32)
    return (pooled, loss, np.float32(nb), np.float32(tp), short_mask)


def kernel(hidden, attention_mask, noise_u, Wq=None, Wk=None, **_unused):
    hidden = np.ascontiguousarray(np.asarray(hidden, dtype=np.float32))
    attention_mask = np.ascontiguousarray(
        np.asarray(attention_mask, dtype=np.float32)
    )
    noise_u = np.ascontiguousarray(np.asarray(noise_u, dtype=np.float32))

    eye = np.eye(D, dtype=np.float32)
    for W in (Wq, Wk):
        if W is not None and not np.array_equal(np.asarray(W), eye):
            return _reference_numpy(
                hidden, attention_mask, noise_u, np.asarray(Wq), np.asarray(Wk)
            )

    from concourse.bass_utils import run_bass_kernel_spmd

    nc = get_nc()
    in_maps = [
        {
            "hidden": hidden[b],
            "noise": noise_u[b],
            "mask": attention_mask[b],
        }
        for b in range(B)
    ]
    res = run_bass_kernel_spmd(nc, in_maps, core_ids=list(range(B)))

    pooled = np.stack([res.results[b]["pooled"] for b in range(B)], axis=0)
    counts = np.array(
        [float(res.results[b]["stats"][0]) for b in range(B)], dtype=np.float64
    )
    nb = counts.sum()
    tp = float(attention_mask.sum(dtype=np.float64))
    loss = _binomial_loss_host(nb, tp)
    short_mask = (
        np.arange(L, dtype=np.float32)[None, :] < counts[:, None]
    ).astype(np.float32)
    return (pooled, loss, np.float32(nb), np.float32(tp), short_mask)


if __name__ == "__main__":
    rng = np.random.default_rng(0)
    h = rng.standard_normal((B, L, D), dtype=np.float32)
    m = np.ones((B, L), np.float32)
    u = rng.random((B, L), dtype=np.float32)
    out = kernel(h, m, u, np.eye(D, np.float32), np.eye(D, np.float32))
    print([np.asarray(o).shape for o in out])


# revision 17
# speedup vs baseline: 1.8867x; 1.4452x over previous
"""Trainium2 Bass kernel for nn_BoundaryPredictor2 (ragged_sequence).

Data-parallel over batch: each of the 8 NeuronCores processes one batch row
(B=8, L=1024, D=1024).

Per-core device pipeline (all fp32, matmuls streamed as float32r):
  1. Load hidden row as 8 tiles [128, 1024] (l on partitions, d on free),
     plus the same tiles shifted by one row (for adjacent-row dots).
  2. Row sum-of-squares via ACT Square+accum; adjacent-row dots via DVE
     scalar_tensor_tensor with accum_out.
  3. In the transposed [8, 128] domain (chunk on partitions, position on
     free axis, so +-1 position shifts are free-axis slices): cos_sim,
     p = clip((1-cos)/2, tiny, 1), and the boundary bit
     hard = (p + clip(u, tiny, 1) > 1) * mask.  This is exactly equivalent
     to the reference's sigmoid((logit(p)+logit(u))/T) > 0.5 route
     (verified bit-identical on the seeded inputs; margin ~2.4e-5).
  4. Segment ids hh1 = exclusive-cumsum(hard) via the native
     tensor_tensor_scan; cross-chunk offsets via a strict-lower-triangular
     [8,8] matmul; PE-transpose back to [128, 8] column layout.
  5. One-hot membership lel[l, s] = (hh1[l] == s) via iota + is_equal
     (only the lower-triangular windows s < 128*(chunk+1) can be nonzero).
  6. pooled_raw[s] = sum_l lel[l, s] * h[l] via PE matmuls (8 output windows
     of 128 segments; chunk lc only feeds windows m <= lc). Segment counts
     come from the same stationary with a ones moving vector.
  7. pooled = pooled_raw * 1/(n + 1e-9), applied during the PSUM->SBUF copy
     on the scalar engine (per-partition scale), then DMA out.

Host side: gather per-core boundary counts, compute the binomial loss with
float64 lgamma, and build short_mask = arange(L) < counts[:, None].
"""

import math
import sys

import numpy as np

try:
    import concourse.bass as bass  # noqa: F401
except ImportError:  # pragma: no cover
    sys.path.insert(0, "/opt/trn_rl_repo")

import concourse.bass as bass
import concourse.mybir as mybir
import concourse.tile as tile
from concourse.masks import make_identity

F32 = mybir.dt.float32
F32R = mybir.dt.float32r
Alu = mybir.AluOpType
Act = mybir.ActivationFunctionType

B, L, D = 8, 1024, 1024
P = 128
NCHUNK = L // P  # 8
PRIOR = 0.2
TINY = float(np.finfo(np.float32).tiny)

_NC_CACHE = {}


def _hoist_multiwaits(nc):
    """Walrus codegen for TRN2 only encodes a single embedded sem-wait per
    instruction ("Too many sync wait commands" otherwise). Split every
    multi-wait sync_info into standalone single-wait EventSemaphore
    instructions placed directly before the instruction on the same engine
    stream (the raw-bass wait_ge pattern)."""
    for f in nc.m.functions:
        for b in f.blocks:
            insts = list(b.instructions)
            if not any(
                i.sync_info is not None and len(i.sync_info.on_wait) > 1
                for i in insts
            ):
                continue
            new = []
            for i in insts:
                si = i.sync_info
                if si is not None and len(si.on_wait) > 1:
                    for k, w in enumerate(si.on_wait):
                        ev = mybir.InstEventSemaphore(
                            name=f"{i.name}-hw{k}",
                            ins=[],
                            outs=[],
                            sync_info=mybir.SyncInfo(on_wait=[w], on_update=[]),
                        )
                        ev.engine = i.engine
                        new.append(ev)
                    i.sync_info = mybir.SyncInfo(
                        on_wait=[], on_update=list(si.on_update)
                    )
                new.append(i)
            b.instructions = new


def build_nc(hoist=True):
    nc = bass.Bass("TRN2", target_bir_lowering=False)

    hidden_d = nc.dram_tensor("hidden", [L, D], F32, kind="ExternalInput")
    noise_d = nc.dram_tensor("noise", [L], F32, kind="ExternalInput")
    mask_d = nc.dram_tensor("mask", [L], F32, kind="ExternalInput")
    pooled_d = nc.dram_tensor("pooled", [L, D], F32, kind="ExternalOutput")
    stats_d = nc.dram_tensor("stats", [1], F32, kind="ExternalOutput")

    with tile.TileContext(nc) as tc:
        with (
            tc.tile_pool(name="hpool", bufs=1) as hpool,
            tc.tile_pool(name="lelpool", bufs=1) as lelpool,
            tc.tile_pool(name="spool", bufs=2) as spool,
            tc.tile_pool(name="small", bufs=1) as small,
            tc.tile_pool(name="outp", bufs=3) as outp,
            tc.tile_pool(name="pmain", bufs=2, space="PSUM") as pmain,
            tc.tile_pool(name="psmall", bufs=2, space="PSUM") as psmall,
            tc.tile_pool(name="ptr", bufs=2, space="PSUM") as ptr,
        ):
            # ---- constants / small buffers ----
            ident = small.tile([P, P], F32, tag="ident")
            make_identity(nc, ident[:])
            ones_col = small.tile([P, 1], F32, tag="ones_col")
            nc.vector.memset(ones_col[:], 1.0)
            # fp32r matmuls need even moving/dst element counts -> 2 cols
            ones2 = small.tile([P, 2], F32, tag="ones2")
            nc.vector.memset(ones2[:], 1.0)
            ones_col_r = small.tile([P, 2], F32R, tag="ones_col_r")
            nc.scalar.activation(ones_col_r[:], ones2[:], Act.Copy, bias=0.0)
            # shift matrix S[k, m] = 1 iff k == m+1 (partition up-shift)
            smat = small.tile([P, P], F32, tag="smat")
            nc.gpsimd.memset(smat[:], 0.0)
            nc.gpsimd.affine_select(
                out=smat[:],
                in_=smat[:],
                compare_op=Alu.not_equal,
                fill=1.0,
                base=-1,
                pattern=[[-1, P]],
                channel_multiplier=1,
            )
            smat_r = small.tile([P, P], F32R, tag="smat_r")
            nc.scalar.activation(smat_r[:], smat[:], Act.Copy, bias=0.0)
            # strict-lower-triangular [8,8]: Lmat[k, m] = 1 if k < m else 0
            lmat = small.tile([NCHUNK, NCHUNK], F32, tag="lmat")
            nc.gpsimd.memset(lmat[:], 0.0)
            nc.gpsimd.affine_select(
                out=lmat[:],
                in_=lmat[:],
                compare_op=Alu.is_ge,
                fill=1.0,
                base=0,
                pattern=[[-1, NCHUNK]],
                channel_multiplier=1,
            )

            ss_cols = small.tile([P, NCHUNK], F32, tag="ss_cols")
            dot_cols = small.tile([P, NCHUNK], F32, tag="dot_cols")
            rt_cols = small.tile([P, NCHUNK], F32, tag="rt_cols")
            rn_cols = small.tile([P, NCHUNK], F32, tag="rn_cols")

            # ---- phase A: loads + row stats (position space, l on parts) ----
            h_tiles = []
            hr_tiles = []
            for lc in range(NCHUNK):
                h_t = hpool.tile([P, D], F32, tag=f"h{lc}")
                nc.sync.dma_start(h_t[:], hidden_d[P * lc : P * (lc + 1), :])
                h_tiles.append(h_t)
                # fp32r-rounded copy for the PE (fp32r streams at full rate)
                h_r = hpool.tile([P, D], F32R, tag=f"hr{lc}")
                nc.scalar.activation(h_r[:], h_t[:], Act.Copy, bias=0.0)
                hr_tiles.append(h_r)

            for lc in range(NCHUNK):
                h_t = h_tiles[lc]
                scr = spool.tile([P, D], F32, tag="scr")
                nc.scalar.activation(
                    scr[:],
                    h_t[:],
                    Act.Square,
                    accum_out=ss_cols[:, lc : lc + 1],
                )
                # adjacent-row dots: shift rows up by one on the PE
                # (psum_sh[m] = h[m+1], row 127 = 0), then fused mult+reduce
                h_r = hr_tiles[lc]
                psum_sh = pmain.tile([P, D], F32, tag="psum")
                nc.tensor.matmul(
                    psum_sh[:, 0:512], smat_r[:], h_r[:, 0:512],
                    start=True, stop=True,
                )
                nc.tensor.matmul(
                    psum_sh[:, 512:1024], smat_r[:], h_r[:, 512:1024],
                    start=True, stop=True,
                )
                scr2 = spool.tile([P, D], F32, tag="scr2")
                nc.vector.tensor_tensor(
                    out=scr2[:], in0=h_t[:], in1=psum_sh[:], op=Alu.mult
                )
                nc.vector.tensor_reduce(
                    out=dot_cols[:, lc : lc + 1], in_=scr2[:],
                    axis=mybir.AxisListType.X, op=Alu.add,
                )

            # chunk-boundary pairs: (row 128c+127, row 128(c+1)), c=0..6
            hrr = hidden_d[:].rearrange("(c p) d -> c p d", p=P)
            bndA = small.tile([NCHUNK, D], F32, tag="bndA")
            bndB = small.tile([NCHUNK, D], F32, tag="bndB")
            nc.sync.dma_start(bndA[:], hrr[:, P - 1, :])
            nc.sync.dma_start(bndB[0 : NCHUNK - 1, :], hrr[1:NCHUNK, 0, :])
            dot_b = small.tile([NCHUNK, 1], F32, tag="dot_b")
            scrb = small.tile([NCHUNK, D], F32, tag="scrb")
            nc.vector.tensor_tensor(
                out=scrb[0 : NCHUNK - 1, :], in0=bndA[0 : NCHUNK - 1, :],
                in1=bndB[0 : NCHUNK - 1, :], op=Alu.mult,
            )
            nc.vector.tensor_reduce(
                out=dot_b[0 : NCHUNK - 1, 0:1], in_=scrb[0 : NCHUNK - 1, :],
                axis=mybir.AxisListType.X, op=Alu.add,
            )
            nc.sync.dma_start(
                dot_cols[P - 1 : P, 0 : NCHUNK - 1], dot_b[0 : NCHUNK - 1, 0:1]
            )

            # rn = 1 / max(sqrt(ss), 1e-12)   (position space [128, 8])
            nc.scalar.sqrt(rt_cols[:], ss_cols[:])
            nc.vector.tensor_scalar(
                out=rt_cols[:], in0=rt_cols[:], scalar1=1e-12, scalar2=None,
                op0=Alu.max,
            )
            nc.vector.reciprocal(rn_cols[:], rt_cols[:])

            # ---- phase B: transposed domain [8, 128] ----
            def transpose_to_T(src_cols, tag):
                ps = ptr.tile([NCHUNK, P], F32, tag="tps")
                nc.tensor.transpose(ps[:], src_cols[:], ident[:])
                dst = small.tile([NCHUNK, P], F32, tag=tag)
                nc.vector.tensor_copy(dst[:], ps[:])
                return dst

            rnT = transpose_to_T(rn_cols, "rnT")
            dotT = transpose_to_T(dot_cols, "dotT")

            # rn_next[i] = rn[i+1] (pair space): free shift + 7 cross elems
            rn_nextT = small.tile([NCHUNK, P], F32, tag="rn_nextT")
            nc.vector.tensor_copy(rn_nextT[:, 0 : P - 1], rnT[:, 1:P])
            nc.vector.memset(rn_nextT[:, P - 1 : P], 1.0)
            nc.sync.dma_start(
                rn_nextT[0 : NCHUNK - 1, P - 1 : P], rnT[1:NCHUNK, 0:1]
            )

            # cos = dot * rn * rn_next  (pair-index space)
            cosT = small.tile([NCHUNK, P], F32, tag="cosT")
            nc.vector.tensor_tensor(
                out=cosT[:], in0=dotT[:], in1=rnT[:], op=Alu.mult
            )
            nc.vector.tensor_tensor(
                out=cosT[:], in0=cosT[:], in1=rn_nextT[:], op=Alu.mult
            )
            # p = clip((1 - cos)/2, tiny, 1.0)
            pclT = small.tile([NCHUNK, P], F32, tag="pclT")
            nc.scalar.activation(
                pclT[:], cosT[:], Act.Copy, bias=0.5, scale=-0.5
            )
            nc.vector.tensor_scalar(
                out=pclT[:], in0=pclT[:], scalar1=TINY, scalar2=1.0,
                op0=Alu.max, op1=Alu.min,
            )

            # u, mask in transposed layout (natural contiguous loads)
            uT = small.tile([NCHUNK, P], F32, tag="uT")
            nc.sync.dma_start(uT[:], noise_d[:].rearrange("(c p) -> c p", p=P))
            maskT = small.tile([NCHUNK, P], F32, tag="maskT")
            nc.sync.dma_start(
                maskT[:], mask_d[:].rearrange("(c p) -> c p", p=P)
            )
            nc.vector.tensor_scalar(
                out=uT[:], in0=uT[:], scalar1=TINY, scalar2=1.0,
                op0=Alu.max, op1=Alu.min,
            )
            # u2[i] = u[i+1] (pair space)
            u2T = small.tile([NCHUNK, P], F32, tag="u2T")
            nc.vector.tensor_copy(u2T[:, 0 : P - 1], uT[:, 1:P])
            nc.vector.memset(u2T[:, P - 1 : P], 0.0)
            nc.sync.dma_start(
                u2T[0 : NCHUNK - 1, P - 1 : P], uT[1:NCHUNK, 0:1]
            )

            # h2[i] = (p[i] + u[i+1] > 1)  -> boundary bit at position i+1
            h2T = small.tile([NCHUNK, P], F32, tag="h2T")
            nc.vector.scalar_tensor_tensor(
                out=h2T[:], in0=pclT[:], scalar=-1.0, in1=u2T[:],
                op0=Alu.add, op1=Alu.add,
            )
            nc.vector.tensor_scalar(
                out=h2T[:], in0=h2T[:], scalar1=0.0, scalar2=None,
                op0=Alu.is_gt,
            )

            # hard[pos] = h2[pos-1] * mask[pos]; position 0 forced boundary
            hardT = small.tile([NCHUNK, P], F32, tag="hardT")
            nc.vector.tensor_copy(hardT[:, 1:P], h2T[:, 0 : P - 1])
            nc.vector.memset(hardT[:, 0:1], 0.0)
            nc.sync.dma_start(
                hardT[1:NCHUNK, 0:1], h2T[0 : NCHUNK - 1, P - 1 : P]
            )
            nc.vector.memset(hardT[0:1, 0:1], 1.0)
            nc.vector.tensor_tensor(
                out=hardT[:], in0=hardT[:], in1=maskT[:], op=Alu.mult
            )

            # cumsum within chunks; hh1 = inclusive - hard (exclusive)
            zerosT = small.tile([NCHUNK, P], F32, tag="zerosT")
            nc.vector.memset(zerosT[:], 0.0)
            csT = small.tile([NCHUNK, P], F32, tag="csT")
            nc.vector.tensor_tensor_scan(
                csT[:], hardT[:], zerosT[:], 0.0, op0=Alu.add, op1=Alu.add
            )
            hh1T = small.tile([NCHUNK, P], F32, tag="hh1T")
            nc.vector.tensor_tensor(
                out=hh1T[:], in0=csT[:], in1=hardT[:], op=Alu.subtract
            )

            # cross-chunk offsets: offs[c] = sum_{k<c} totals[k]
            offs_ps = ptr.tile([NCHUNK, 1], F32, tag="tps")
            nc.tensor.matmul(
                offs_ps[:], lmat[:], csT[:, P - 1 : P], start=True, stop=True
            )
            offs_col = small.tile([NCHUNK, 1], F32, tag="offs_col")
            nc.vector.tensor_copy(offs_col[:], offs_ps[:])
            hh1gT = small.tile([NCHUNK, P], F32, tag="hh1gT")
            nc.vector.tensor_scalar(
                out=hh1gT[:], in0=hh1T[:], scalar1=offs_col[:], scalar2=None,
                op0=Alu.add,
            )

            # total boundary count -> stats
            nb_ps = ptr.tile([1, 1], F32, tag="tps")
            nc.tensor.matmul(
                nb_ps[:], ones_col[0:NCHUNK, 0:1], csT[:, P - 1 : P],
                start=True, stop=True,
            )
            nbt = small.tile([1, 1], F32, tag="nbt")
            nc.vector.tensor_copy(nbt[:], nb_ps[:])
            nc.sync.dma_start(stats_d[:], nbt[:])

            # back to column layout [128, 8]
            hh1_ps = ptr.tile([P, NCHUNK], F32, tag="tps")
            nc.tensor.transpose(
                hh1_ps[:], hh1gT[:], ident[0:NCHUNK, 0:NCHUNK]
            )
            hh1g = small.tile([P, NCHUNK], F32, tag="hh1g")
            nc.vector.tensor_copy(hh1g[:], hh1_ps[:])

            # ---- phase C: one-hot membership + pooling matmuls ----
            iota_b = small.tile([P, D], F32, tag="iota_b")
            nc.gpsimd.iota(
                iota_b[:],
                pattern=[[1, D]],
                base=0,
                channel_multiplier=0,
                allow_small_or_imprecise_dtypes=True,
            )
            lel_tiles = []
            for lc in range(NCHUNK):
                W = P * (lc + 1)
                lel_t = lelpool.tile([P, L], F32R, tag=f"lel{lc}")
                nc.vector.tensor_scalar(
                    out=lel_t[:, 0:W], in0=iota_b[:, 0:W],
                    scalar1=hh1g[:, lc : lc + 1], scalar2=None,
                    op0=Alu.is_equal,
                )
                lel_tiles.append(lel_t)

            for m in range(NCHUNK):
                psum = pmain.tile([P, D], F32, tag="psum")
                npsum = psmall.tile([P, 2], F32, tag="npsum")
                for lc in range(m, NCHUNK):
                    st = lc == m
                    sp = lc == NCHUNK - 1
                    lelT = lel_tiles[lc][:, P * m : P * (m + 1)]
                    h_r = hr_tiles[lc][:]
                    nc.tensor.matmul(
                        psum[:, 0:512], lelT, h_r[:, 0:512], start=st, stop=sp
                    )
                    nc.tensor.matmul(
                        psum[:, 512:1024], lelT, h_r[:, 512:1024],
                        start=st, stop=sp,
                    )
                    nc.tensor.matmul(
                        npsum[:], lelT, ones_col_r[:], start=st, stop=sp,
                    )
                ncol = small.tile([P, 1], F32, tag="ncol")
                nc.vector.tensor_scalar(
                    out=ncol[:], in0=npsum[:, 0:1], scalar1=1e-9, scalar2=None,
                    op0=Alu.add,
                )
                rcol = small.tile([P, 1], F32, tag="rcol")
                nc.vector.reciprocal(rcol[:], ncol[:])
                outt = outp.tile([P, D], F32, tag="outt")
                nc.scalar.activation(
                    outt[:], psum[:], Act.Copy, bias=0.0, scale=rcol[:]
                )
                nc.sync.dma_start(pooled_d[P * m : P * (m + 1), :], outt[:])

    if hoist:
        _hoist_multiwaits(nc)
    return nc


def get_nc(hoist=True):
    key = ("nc", hoist)
    if key not in _NC_CACHE:
        _NC_CACHE[key] = build_nc(hoist=hoist)
    return _NC_CACHE[key]


def _binomial_loss_host(n, total, prior=PRIOR):
    n = float(n)
    total = float(total)
    logp = (
        math.lgamma(total + 1.0)
        - math.lgamma(n + 1.0)
        - math.lgamma(total - n + 1.0)
        + n * math.log(prior)
        + (total - n) * math.log1p(-prior)
    )
    return np.float32(-logp / total)


def _reference_numpy(hidden, attention_mask, noise_u, Wq, Wk):
    """Pure-numpy replica of the oracle; only used if Wq/Wk are not identity
    (the shipped problem always uses identity projections)."""
    h = hidden.astype(np.float64)
    n = np.maximum(np.linalg.norm(h, axis=-1, keepdims=True), 1e-12)
    hn = h / n
    q = hn[:, :-1] @ Wq.T.astype(np.float64)
    k = hn[:, 1:] @ Wk.T.astype(np.float64)
    cos = np.einsum("bld,bld->bl", q, k)
    probs = np.clip((1.0 - cos) / 2.0, 0.0, 1.0)
    probs = np.concatenate([np.ones((B, 1)), probs], axis=1)
    tiny = float(np.finfo(np.float32).tiny)
    p = np.clip(probs, tiny, 1.0 - 1e-30)
    u = np.clip(noise_u.astype(np.float64), tiny, 1.0 - 1e-12)
    logits = np.log(p) - np.log1p(-p) + np.log(u) - np.log1p(-u)
    hard = (logits > 0.0).astype(np.float32) * attention_mask
    hh1 = np.cumsum(hard, axis=1) - hard
    s_range = np.arange(L, dtype=np.float32)
    pooled = np.zeros((B, L, D), np.float32)
    for b in range(B):
        lel = (s_range[None, :] == hh1[b][:, None]).astype(np.float32)
        bar = lel / (lel.sum(axis=0, keepdims=True) + 1e-9)
        pooled[b] = (hidden[b].T @ bar).T
    nb = hard.sum()
    tp = attention_mask.sum()
    loss = _binomial_loss_host(nb, tp)
    counts = (hard == 1.0).sum(axis=1)
    short_mask = (np.arange(L)[None, :] < counts[:, None]).astype(np.float32)
    return (pooled, loss, np.float32(nb), np.float32(tp), short_mask)


def kernel(hidden, attention_mask, noise_u, Wq=None, Wk=None, **_unused):
    hidden = np.ascontiguousarray(np.asarray(hidden, dtype=np.float32))
    attention_mask = np.ascontiguousarray(
        np.asarray(attention_mask, dtype=np.float32)
    )
    noise_u = np.ascontiguousarray(np.asarray(noise_u, dtype=np.float32))

    eye = np.eye(D, dtype=np.float32)
    for W in (Wq, Wk):
        if W is not None and not np.array_equal(np.asarray(W), eye):
            return _reference_numpy(
                hidden, attention_mask, noise_u, np.asarray(Wq), np.asarray(Wk)
            )

    from concourse.bass_utils import run_bass_kernel_spmd

    nc = get_nc()
    in_maps = [
        {
            "hidden": hidden[b],
            "noise": noise_u[b],
            "mask": attention_mask[b],
        }
        for b in range(B)
    ]
    res = run_bass_kernel_spmd(nc, in_maps, core_ids=list(range(B)))

    pooled = np.stack([res.results[b]["pooled"] for b in range(B)], axis=0)
    counts = np.array(
        [float(res.results[b]["stats"][0]) for b in range(B)], dtype=np.float64
    )
    nb = counts.sum()
    tp = float(attention_mask.sum(dtype=np.float64))
    loss = _binomial_loss_host(nb, tp)
    short_mask = (
        np.arange(L, dtype=np.float32)[None, :] < counts[:, None]
    ).astype(np.float32)
    return (pooled, loss, np.float32(nb), np.float32(tp), short_mask)


if __name__ == "__main__":
    rng = np.random.default_rng(0)
    h = rng.standard_normal((B, L, D), dtype=np.float32)
    m = np.ones((B, L), np.float32)
    u = rng.random((B, L), dtype=np.float32)
    out = kernel(h, m, u, np.eye(D, np.float32), np.eye(D, np.float32))
    print([np.asarray(o).shape for o in out])
